# revision 18
# baseline (speedup 1.0000x reference)
"""Trainium2 Bass kernel for nn_MixedAttnHeadEmbed_82076825027210.

Computes, per batch element:
    out = sum over h in {4, 8, 12} of CausalAttention(Q_mix_h, K_mix_h, V_mix_h)
where Q/K/V_mix_h are weighted mixtures (9 scalar weights) of head-sliced
views of x's q/k/v channel groups, padded per head to hd = 768/h.

Sharding: data-parallel over batch B=8 across the 8 NeuronCores (one batch
element per core); the 9 mixture weights are baked into the compiled program
as immediates.

Per-core plan (T=1024 tokens, bf16 compute, fp32 accumulation):
  1. Six SWDGE cast-DMAs load x [1024, 2304] f32 -> SBUF bf16 in half-head
     column chunks so mixing starts as soon as the first chunk lands.
  2. Per config and per half of the heads, DVE builds mixed Q/K naturals
     (tensor_scalar at 4x + tensor_tensor adds at 2x -- scalar_tensor_tensor
     runs at 1x) and V_aug with a ones column per head for the softmax
     denominator.
  3. Each half bounces through DRAM and returns via HWDGE DMA-transpose as
     Q^T/K^T [d, T] bf16 matmul operands; Q uses the SP ring and K the ACT
     ring (per-ring FIFO makes the DRAM RAW ordering real), so the two
     tensors' DMAs overlap and attention pass 0 starts after half the work.
  4. Attention per config, per 512-query block, per half-of-heads pass:
     S^T = K_mix Q_mix^T blockwise on PE (causal blocks only; diagonal
     blocks masked by one extra ustrict x negi matmul per head), exp on ACT
     with the softmax scale folded in (max-subtraction skipped: |S*scale|
     is small), then Y = P V_aug accumulated *natural* (queries on
     partitions) in PSUM with P^T tiles as the stationary operand -- the
     ones-column lands the denominator l as an extra output column. PSUM
     start=True marks a whole 2KB zero region, so only the first matmul
     into each Y bank carries it (with explicit ordering deps).
  5. Per query tile: DVE reciprocal of l, then POOL scalar_tensor_tensor
     normalize-and-accumulate from PSUM into the fp32 output accumulator;
     the result streams out in per-query-block DMAs as configs finish.
"""

import math

import numpy as np

import concourse.bass as bass
import concourse.bacc as bacc
import concourse.tile as tile
from concourse import mybir
from concourse.bass_utils import run_bass_kernel_spmd
from concourse.tile import add_dep_helper

F32 = mybir.dt.float32
BF16 = mybir.dt.bfloat16
ALU = mybir.AluOpType
ACTF = mybir.ActivationFunctionType

T = 1024
NT = 8  # token tiles of 128
E = 768
CIN = 3 * E
N_HEAD_LIST = (4, 8, 12)
N_CORES = 8
MASK_NEG = -3000.0  # additive pre-scale mask; exp(scale*MASK_NEG) == 0


def _pw(h):
    """Per-head column pitch in the natural mixed layout; h=8 pads 96 -> 128
    so every transposed head starts at a legal matmul base partition."""
    return 128 if h == 8 else E // h


def _dchunks(h):
    """Per head: contraction (d) row ranges in the transposed layout, split
    at 128-row tile boundaries."""
    hd = E // h
    pitch = _pw(h)
    out = []
    for i in range(h):
        a, b = i * pitch, i * pitch + hd
        chunks = []
        while a < b:
            nxt = min(b, (a // 128 + 1) * 128)
            chunks.append((a, nxt))
            a = nxt
        out.append(chunks)
    return out


def _build_program(W):
    """W: numpy [9] f32 mixture weights. Returns compiled Bacc program."""
    nc = bacc.Bacc(
        "TRN2", target_bir_lowering=False, debug=False, num_devices=N_CORES
    )
    x_in = nc.dram_tensor("x", [T, CIN], F32, kind="ExternalInput").ap()
    out_d = nc.dram_tensor("out", [T, E], F32, kind="ExternalOutput").ap()
    qk_dram = [
        [
            nc.dram_tensor(
                f"qkb_{ci}_{ti}", [T, N_HEAD_LIST[ci] * _pw(N_HEAD_LIST[ci])],
                BF16,
            ).ap()
            for ti in range(2)
        ]
        for ci in range(3)
    ]

    with tile.TileContext(nc) as tc:
        _emit(tc, x_in, out_d, qk_dram, W)
    nc.compile()
    return nc


def _mix_half(nc, W, ci, out_ap, xsrc, tmps, e_list, h2, add_eng=None):
    """Mixed half-tensor: out[:, :, i, 0:e/h] (+)= w_e * xsrc_e per e.
    tensor_scalar (4x) for the largest e, then ts into tmp + tensor_tensor
    add (2x) for the rest -- scalar_tensor_tensor would run at 1x. The adds
    can run on POOL (add_eng) to offload the DVE."""
    add_eng = add_eng or nc.vector
    for idx, (k, e, hde) in enumerate(e_list):
        w = float(W[3 * ci + k])
        in0 = xsrc(e, hde)
        if idx == 0:
            nc.vector.tensor_scalar(
                out_ap(hde), in0, w, None, ALU.mult
            )
        else:
            tview = tmps[idx % len(tmps)].rearrange(
                "p a (h d) -> p a h d", h=h2
            )
            tv = tview[:, :, :, 0:hde]
            nc.vector.tensor_scalar(tv, in0, w, None, ALU.mult)
            add_eng.tensor_tensor(out_ap(hde), tv, out_ap(hde), ALU.add)


def _emit(tc, x_in, out_d, qk_dram, W):
    nc = tc.nc
    with (
        tc.tile_pool(name="consts", bufs=1) as consts,
        tc.tile_pool(name="xbf", bufs=1) as xbf_pool,
        tc.tile_pool(name="nat", bufs=2) as nat_pool,
        tc.tile_pool(name="tmp", bufs=1) as tmp_pool,
        tc.tile_pool(name="qkt", bufs=2) as qkt_pool,
        tc.tile_pool(name="vaug", bufs=3) as vaug_pool,
        tc.tile_pool(name="pt", bufs=4) as pt_pool,
        tc.tile_pool(name="small", bufs=4) as small_pool,
        tc.tile_pool(name="oacc", bufs=1) as oacc_pool,
        tc.tile_pool(name="stage", bufs=2, space="PSUM") as stage_pool,
        tc.tile_pool(name="ypsum", bufs=4, space="PSUM") as ypsum_pool,
    ):
        # ---- constants: strict-upper selector and MASK_NEG * I ----------
        ustrict = consts.tile([128, 128], BF16)
        nc.gpsimd.memset(ustrict, 1.0)
        nc.gpsimd.affine_select(
            out=ustrict, in_=ustrict, compare_op=ALU.is_gt, fill=0.0,
            base=0, pattern=[[1, 128]], channel_multiplier=-1,
        )
        negi = consts.tile([128, 128], BF16)
        nc.gpsimd.memset(negi, 0.0)
        nc.gpsimd.affine_select(
            out=negi, in_=negi, compare_op=ALU.not_equal, fill=MASK_NEG,
            base=0, pattern=[[-1, 128]], channel_multiplier=1,
        )

        # ---- load x in half-head column chunks, cast to bf16 ------------
        # order: Q half0, K half0, V half0, Q half1, K half1, V half1
        xbf = xbf_pool.tile([128, NT, CIN], BF16)
        for half in range(2):
            for third in range(3):
                c0 = third * E + half * (E // 2)
                nc.gpsimd.dma_start(
                    out=xbf[:, :, c0 : c0 + E // 2],
                    in_=x_in[:, c0 : c0 + E // 2].rearrange(
                        "(a p) c -> p a c", p=128
                    ),
                )

        oacc = oacc_pool.tile([128, NT, E], F32)

        state = {}

        # weight order in W: for cfg ci, e in (384, 576, 768): W[3*ci + idx]
        def mix_config(ci):
            # generator: yields after each (half, tensor) piece so the
            # driver can interleave DVE mixing with the previous config's
            # attention normalizes (DVE executes in emission order)
            h = N_HEAD_LIST[ci]
            hd = E // h
            pw = _pw(h)
            h2 = h // 2
            scale = 1.0 / math.sqrt(hd)
            dchunks = _dchunks(h)
            ndt = h * pw // 128
            ndt2 = ndt // 2
            e_list = [(2, 768, hd), (1, 576, 576 // h), (0, 384, 384 // h)]

            # ---- mix + bounce + transpose per half of the heads ---------
            qkt = []
            vaug = vaug_pool.tile([128, NT, h, hd + 1], BF16, tag="vaug")
            state_set = False
            for tensor_idx in range(2):
                tl = qkt_pool.tile(
                    [128, ndt, T], BF16, tag="qkt", bufs=4,
                    name=f"qkt{ci}{tensor_idx}",
                )
                qkt.append(tl)
            tmp = tmp_pool.tile([128, NT, 288], BF16, tag="tmp")
            tmpb = tmp_pool.tile([128, NT, 288], BF16, tag="tmpb")
            state[ci] = (qkt, vaug)
            for half in range(2):
                hsl = slice(half * h2, (half + 1) * h2)
                for tensor_idx in range(2):  # 0=Q (SP ring) 1=K (ACT ring)
                    base = tensor_idx * E
                    nat = nat_pool.tile(
                        [128, NT, h2, pw], BF16, tag="nat"
                    )
                    if pw > hd:
                        nc.vector.memset(nat[:, :, :, hd:pw], 0.0)

                    def xsrc(e, hde, base=base, half=half):
                        sl = xbf[
                            :, :,
                            base + half * (e // 2) : base + (half + 1) * (e // 2),
                        ]
                        return sl.rearrange("p a (h d) -> p a h d", h=h2)

                    def out_ap(hde, nat=nat):
                        return nat[:, :, :, 0:hde]

                    _mix_half(nc, W, ci, out_ap, xsrc, (tmp, tmpb), e_list, h2)

                    eng = nc.sync  # single HWDGE ring (dual-ring raced)
                    w0 = half * h2 * pw
                    wr = eng.dma_start(
                        out=qk_dram[ci][tensor_idx][
                            :, w0 : w0 + h2 * pw
                        ].rearrange("(a p) w -> p a w", p=128),
                        in_=nat[:, :, :, :],
                    )
                    for dt_ in range(half * ndt2, (half + 1) * ndt2):
                        rd = eng.dma_start(
                            out=qkt[tensor_idx][:, dt_, :],
                            in_=qk_dram[ci][tensor_idx][
                                :, dt_ * 128 : (dt_ + 1) * 128
                            ],
                            transpose=True,
                        )
                        add_dep_helper(
                            rd.ins, wr.ins, sync=True,
                            reason="dram bounce raw",
                        )
                    yield

                # V_aug for this half
                nc.vector.memset(vaug[:, :, hsl, hd : hd + 1], 1.0)

                def vsrc(e, hde, half=half):
                    sl = xbf[
                        :, :,
                        2 * E + half * (e // 2) : 2 * E + (half + 1) * (e // 2),
                    ]
                    return sl.rearrange("p a (h d) -> p a h d", h=h2)

                def vout(hde, hsl=hsl):
                    return vaug[:, :, hsl, 0:hde]

                _mix_half(nc, W, ci, vout, vsrc, (tmp, tmpb), e_list, h2,
                           add_eng=nc.gpsimd)
                yield

        def attn_config(ci):
            h = N_HEAD_LIST[ci]
            hd = E // h
            h2 = h // 2
            scale = 1.0 / math.sqrt(hd)
            dchunks = _dchunks(h)
            qkt, vaug = state.pop(ci)
            qt, kt = qkt

            # ---- attention ---------------------------------------------
            for s in range(2):
                ntk = 4 * s + 4
                for hf in range(2):
                    pheads = list(range(hf * h2, (hf + 1) * h2))
                    nh = h2
                    groups = [pheads[i : i + 2] for i in range(0, nh, 2)]
                    yts = [
                        ypsum_pool.tile(
                            [128, nh, hd + 1], F32, tag="y", name=f"yt{qt_}"
                        )
                        for qt_ in range(4)
                    ]
                    # One accumulation start per PSUM bank: start=True marks
                    # the whole 2KB zero region pending-zero, so only the
                    # first matmul in each Y bank carries it; later heads'
                    # first writes overwrite via the pending-zero bytes.
                    y_first = [None] * 4
                    for tk in range(ntk):
                        lo = max(0, tk * 128 - s * 512)
                        diag = tk >= 4 * s
                        dlo = tk * 128 - s * 512
                        for g in groups:
                            stage = stage_pool.tile(
                                [128, 2, 512], F32, tag="stage"
                            )
                            for j, head in enumerate(g):
                                chunks = dchunks[head]
                                n_mm = len(chunks) + (1 if diag else 0)
                                for mi, (a, b) in enumerate(chunks):
                                    nc.tensor.matmul(
                                        out=stage[:, j, lo:512],
                                        lhsT=kt[
                                            a % 128 : a % 128 + (b - a),
                                            a // 128,
                                            tk * 128 : (tk + 1) * 128,
                                        ],
                                        rhs=qt[
                                            a % 128 : a % 128 + (b - a),
                                            a // 128,
                                            s * 512 + lo : (s + 1) * 512,
                                        ],
                                        start=(mi == 0),
                                        stop=(mi == n_mm - 1),
                                    )
                                if diag:
                                    nc.tensor.matmul(
                                        out=stage[:, j, dlo : dlo + 128],
                                        lhsT=ustrict[:, :],
                                        rhs=negi[:, :],
                                        start=False,
                                        stop=True,
                                    )
                            ptl = pt_pool.tile([128, 2, 512], BF16, tag="pt")
                            nc.scalar.activation(
                                out=ptl[:, 0:2, lo:512],
                                in_=stage[:, 0:2, lo:512],
                                func=ACTF.Exp,
                                scale=scale,
                            )
                            for qt_ in range(4):
                                qtg = 4 * s + qt_
                                if qtg < tk:
                                    continue
                                for j, head in enumerate(g):
                                    jp = head - hf * h2
                                    is_start = (
                                        tk == 0 and y_first[qt_] is None
                                    )
                                    mm = nc.tensor.matmul(
                                        out=yts[qt_][:, jp, :],
                                        lhsT=ptl[
                                            :, j, qt_ * 128 : (qt_ + 1) * 128
                                        ],
                                        rhs=vaug[:, tk, head, :],
                                        start=is_start,
                                        stop=(tk == qtg and jp == nh - 1),
                                    )
                                    if is_start:
                                        y_first[qt_] = mm
                                    elif tk == 0:
                                        add_dep_helper(
                                            mm.ins,
                                            y_first[qt_].ins,
                                            reason="psum zero-region order",
                                        )
                        # normalize query tile tk-4s the moment its
                        # accumulation stops, freeing its Y bank early so
                        # the next pass's PV is not head-of-line blocked
                        qt_ = tk - 4 * s
                        if 0 <= qt_ < 4:
                            tqg = 4 * s + qt_
                            lrow = small_pool.tile([128, 6], F32, tag="lrow")
                            rec = small_pool.tile([128, 6], F32, tag="rec")
                            nc.vector.tensor_copy(
                                lrow[:, 0:nh], yts[qt_][:, :, hd]
                            )
                            nc.vector.reciprocal(rec[:, 0:nh], lrow[:, 0:nh])
                            for jp, head in enumerate(pheads):
                                dst = oacc[
                                    :, tqg, head * hd : head * hd + hd
                                ]
                                if ci == 0:
                                    nc.vector.tensor_scalar(
                                        dst, yts[qt_][:, jp, 0:hd],
                                        rec[:, jp : jp + 1], None, ALU.mult,
                                    )
                                else:
                                    nc.vector.scalar_tensor_tensor(
                                        out=dst,
                                        in0=yts[qt_][:, jp, 0:hd],
                                        scalar=rec[:, jp : jp + 1],
                                        in1=dst,
                                        op0=ALU.mult,
                                        op1=ALU.add,
                                    )
                            if ci == 2 and hf == 1:
                                # this query tile is final: stream out
                                nc.sync.dma_start(
                                    out=out_d[
                                        tqg * 128 : (tqg + 1) * 128, :
                                    ],
                                    in_=oacc[:, tqg, :],
                                )
                    yield

        def drive(gen, n=None):
            done = 0
            for _ in gen if n is None else range(n):
                if n is not None:
                    try:
                        next(gen)
                    except StopIteration:
                        return None
                done += 1
            return gen

        for _ in mix_config(0):
            pass
        m_next = mix_config(1)
        for ci in range(3):
            a = attn_config(ci)
            while True:
                try:
                    next(a)
                except StopIteration:
                    break
                # advance next config's mixing by ~2 pieces per pass
                if m_next is not None:
                    for _ in range(2):
                        try:
                            next(m_next)
                        except StopIteration:
                            m_next = None
                            break
            # drain remaining mix pieces before the next attention
            if m_next is not None:
                for _ in m_next:
                    pass
            m_next = mix_config(2) if ci == 0 else None


_PROGRAM_CACHE = {}


def _get_program(W):
    key = np.asarray(W, dtype=np.float32).tobytes()
    if key not in _PROGRAM_CACHE:
        _PROGRAM_CACHE[key] = _build_program(np.asarray(W, dtype=np.float32))
    return _PROGRAM_CACHE[key]


def kernel(x, weights):
    """x: [8, 1024, 2304] f32; weights: [9] f32 -> [8, 1024, 768] f32."""
    x = np.asarray(x, dtype=np.float32)
    weights = np.asarray(weights, dtype=np.float32)
    assert x.shape == (N_CORES, T, CIN), x.shape
    nc = _get_program(weights)
    in_maps = [{"x": np.ascontiguousarray(x[c])} for c in range(N_CORES)]
    res = run_bass_kernel_spmd(nc, in_maps, list(range(N_CORES)))
    return np.stack([res.results[c]["out"] for c in range(N_CORES)], axis=0)
